# revision 36
# baseline (speedup 1.0000x reference)
"""Trainium2 Bass kernel for nn_DecoupledEmbeddingModel (B=2048, D=512, C=110, H=1024, V=50257).

Strategy: 8 cores, paired. Each core runs the front MLP stack for its own 256
rows, then the pair exchanges final activations (pairwise AllGather, 0.5 MB)
and each pair member computes the tied lm_head for all 512 pair rows against
HALF the vocab (25600 entries). This halves the dominant lm weight streaming
vs. pure data-parallel while keeping the PE tensor engine the bottleneck
(dense matmul stream -> full 2.4 GHz p-state).

The lm loop covers the exchange latency with a phase A that computes the
core's OWN rows for the first PRE_K vocab tiles of BOTH halves (pair symmetry
makes coverage exact with zero redundancy), then phase B computes all four
pair row tiles for the remaining own-half tiles.

The delta-rule fast-weight path (x_hat / dR / temporal) scales with h_prev,
which setup_inputs always provides as zeros; a full-math host fallback
handles the general case.
"""

import math
import sys

sys.path.insert(0, "/opt/trn_rl_repo")

import numpy as np

import concourse.bass as bass
import concourse.tile as tile
from concourse import bacc, mybir
from concourse.bass_utils import run_bass_kernel_spmd
from concourse.masks import make_identity

AF = mybir.ActivationFunctionType
ALU = mybir.AluOpType
f32 = mybir.dt.float32
f32r = mybir.dt.float32r
bf16 = mybir.dt.bfloat16
P = 128

# Model dims
V, D, C, H = 50257, 512, 110, 1024
B = 2048
NCORES = 8

# Config knobs
LM_DT = mybir.dt.bfloat16
VT = 512                      # vocab tile width
LMW_BUFS = 4
LS_BUFS = 2
PRE_K = 4                     # own-rows-only prefix (covers exchange latency)

# Vocab padded to pairs of 512 tiles, split across 2 cores
NV2_FULL = math.ceil(V / (2 * VT))    # 50 double-tiles total
NV2C = NV2_FULL // 2                  # 25 double-tiles per core
VP_FULL = NV2_FULL * 2 * VT           # 51200
VC = NV2C * 2 * VT                    # 25600 vocab per core
BB_FULL = B // NCORES                 # 256 own rows
RT_N = BB_FULL // P                   # 2 own row tiles
RT_ALL = 2 * RT_N                     # 4 row tiles in lm (own + partner)


def np_dt(dt):
    return np.dtype(mybir.dt.np(dt))


# ----------------------------------------------------------------------------
# Device program
# ----------------------------------------------------------------------------

def build_nc(nv2c=NV2C, pre_k=PRE_K):
    nc = bacc.Bacc("TRN2", target_bir_lowering=False, debug=False,
                   enable_asserts=False, num_devices=NCORES)

    a = {}
    def din(name, shape, dt=f32):
        a[name] = nc.dram_tensor(name, list(shape), dt, kind="ExternalInput").ap()

    din("tok", [BB_FULL, 1], mybir.dt.int32)
    din("etab", [V, D])
    # wv tiles: [0..nv2c-1] own vocab half, [nv2c..nv2c+pre_k-1] prefix of the
    # OTHER half (phase A covers own rows for both halves' prefixes; pair
    # symmetry makes coverage exact with no redundancy)
    din("wv", [nv2c + pre_k, P, 8, VT], LM_DT)
    din("cp_w", [P, 4, C], f32r)
    din("cp_b", [C])
    din("h1_w", [P, H], f32r)      # rows: 0..109 = h_w1.T, 110 = h_b1, rest 0
    din("h2_w", [P, 8, C], bf16)
    din("h2_b", [C])
    din("up_w", [P, D], f32r)      # rows: 0..109 = up_w.T, 110 = up_b, rest 0
    din("h1_b", [H]); din("up_b", [D])
    din("rn_g", [D]); din("rn_b", [D])
    din("p1_w", [P, 16, H], bf16)  # fusion-LN g/b folded in
    din("p1_b", [H])
    din("p2_w", [P, 8, D], bf16)   # pln-LN g/b folded in
    din("p2_b", [D])
    # exchange buffers (DRAM): own packed outT -> pair AllGather
    dscr = nc.dram_tensor("dscr", [P, 1], f32r).ap()
    gin = nc.dram_tensor("gin", [P, RT_N * 4 * P], bf16).ap()
    gbuf = nc.dram_tensor("gbuf", [2, P, RT_N * 4 * P], bf16).ap()
    # outA: own rows x prefix tiles of both halves; outB: all pair rows x rest
    outA_ap = nc.dram_tensor("outA", [RT_N * P, 2 * pre_k * 2 * VT], bf16,
                             kind="ExternalOutput").ap()
    outB_ap = nc.dram_tensor("outB", [RT_ALL * P, (nv2c - pre_k) * 2 * VT],
                             bf16, kind="ExternalOutput").ap()

    with tile.TileContext(nc) as tc:
        _program(tc, a, dscr, gin, gbuf, outA_ap, outB_ap, nv2c, pre_k)
    nc.compile()
    return nc


def _pre_tile_idx(ai, nv2c, pre_k):
    """Phase-A tile order: own-half prefix then other-half prefix."""
    hx, i = divmod(ai, pre_k)
    return i if hx == 0 else nv2c + i


def _program(tc, a, dscr, gin, gbuf, outA_ap, outB_ap, nv2c, pre_k):
    nc = tc.nc
    from contextlib import ExitStack
    ctx = ExitStack()
    with ctx:
        consts = ctx.enter_context(tc.tile_pool(name="consts", bufs=1))
        work = ctx.enter_context(tc.tile_pool(name="work", bufs=1))
        lmw = ctx.enter_context(tc.tile_pool(name="lmw", bufs=LMW_BUFS))
        lms = ctx.enter_context(tc.tile_pool(name="lms", bufs=LS_BUFS))
        ps_acc = ctx.enter_context(tc.tile_pool(name="ps_acc", bufs=1, space="PSUM"))
        ps_t = ctx.enter_context(tc.tile_pool(name="ps_t", bufs=2, space="PSUM"))
        ps_lm = ctx.enter_context(tc.tile_pool(name="ps_lm", bufs=2, space="PSUM"))

        ident = consts.tile([P, P], f32)
        make_identity(nc, ident[:])
        identr = consts.tile([P, P], f32r)
        nc.vector.tensor_copy(out=identr[:], in_=ident[:])
        identb = consts.tile([P, P], bf16)
        nc.vector.tensor_copy(out=identb[:], in_=ident[:])
        eps5 = consts.tile([P, 1], f32)
        nc.vector.memset(eps5[:], 1e-5)

        def brep(name, n):
            t = consts.tile([P, n], f32, tag=f"br_{name}")
            src = a[name]
            bsrc = bass.AP(tensor=src.tensor, offset=src.offset,
                           ap=[[0, P]] + list(src.ap))
            nc.gpsimd.dma_start(out=t[:], in_=bsrc)
            return t

        # cin/cout tiles: zero the pad columns once, off the critical path
        cin_tiles, cout_tiles = [], []
        for rt in range(RT_N):
            cin = work.tile([P, P], f32, tag=f"cin{rt}")
            nc.vector.memset(cin[:, C:], 0.0)
            cin_tiles.append(cin)
            cout = work.tile([P, P], f32, tag=f"cout{rt}")
            nc.vector.memset(cout[:, C:], 0.0)
            cout_tiles.append(cout)

        # token fetch + embedding gathers first: they gate the front and do
        # not contend with the big weight loads
        emb_tiles = []
        for rt in range(RT_N):
            tokt = work.tile([P, 1], mybir.dt.int32, tag=f"tok{rt}")
            nc.sync.dma_start(out=tokt[:], in_=a["tok"][rt * P:(rt + 1) * P, :])
            emb = work.tile([P, D], f32r, tag=f"emb{rt}")
            nc.gpsimd.indirect_dma_start(
                out=emb[:], out_offset=None,
                in_=a["etab"][:],
                in_offset=bass.IndirectOffsetOnAxis(ap=tokt[:, :1], axis=0),
            )
            emb_tiles.append(emb)
        # dummy sync-queue DMA reading the last gather: blocks the big weight
        # loads below from hogging the (exclusive) DMA engines before the
        # latency-critical gathers have landed
        nc.sync.dma_start(out=dscr, in_=emb_tiles[-1][:, 0:1])

        cp_b = brep("cp_b", C)
        h2_b = brep("h2_b", C)
        h1_b = brep("h1_b", H)
        up_b = brep("up_b", D)
        rn_g = brep("rn_g", D); rn_b = brep("rn_b", D)
        p1_b = brep("p1_b", H)
        p2_b = brep("p2_b", D)

        def wload(name, shape, dt=f32r):
            t = consts.tile(list(shape), dt, tag=f"w_{name}")
            nc.sync.dma_start(out=t[:], in_=a[name])
            return t

        cp_w = wload("cp_w", [P, 4, C])
        h1_w = wload("h1_w", [P, H])
        h2_w = wload("h2_w", [P, 8, C], bf16)
        up_w = wload("up_w", [P, D])
        p2_w = wload("p2_w", [P, 8, D], bf16)
        p1_w = wload("p1_w", [P, 16, H], bf16)
        # prefetch the first lm weight tiles while the front runs (the sync
        # queue later blocks on front-dependent DMAs)
        wt_pre = []
        for i in range(LMW_BUFS):
            wt = lmw.tile([P, 8, VT], LM_DT, tag="wt")
            nc.sync.dma_start(out=wt[:], in_=a["wv"][_pre_tile_idx(i, nv2c, pre_k)])
            wt_pre.append(wt)

        # packed own outT (exchange payload + own lm lhsT): [p, rt, k, m]
        outTex = consts.tile([P, RT_N, 4, P], LM_DT, tag="outTex", name="outTex")
        # both pair members' outT read back from the gather: [p, slot*rt, k, m]
        pouT = consts.tile([P, RT_ALL, 4, P], LM_DT, tag="pouT", name="pouT")

        # ---------------- helpers ----------------
        def l2norm_scalar(x, n, rt, tag):
            """Return [P,1] tile = 1/||x||_2 (no clamp; rows never zero)."""
            sq = work.tile([P, n], f32, tag=f"sq{n}_{rt}")
            ss = work.tile([P, 1], f32, tag=f"ss_{tag}{rt}")
            nc.scalar.activation(out=sq[:], in_=x, func=AF.Square,
                                 accum_out=ss[:])
            nc.scalar.activation(out=ss[:], in_=ss[:], func=AF.Sqrt)
            nc.vector.reciprocal(ss[:], ss[:])
            return ss

        def layernorm_inplace(x, n, rt, g=None, b=None):
            nsub = n // 512
            st = work.tile([P, nsub, 6], f32, tag=f"lnst{rt}")
            for i in range(nsub):
                nc.vector.bn_stats(out=st[:, i, :], in_=x[:, i * 512:(i + 1) * 512])
            mv = work.tile([P, 2], f32, tag=f"lnmv{rt}")
            nc.vector.bn_aggr(out=mv[:], in_=st[:])
            nc.scalar.activation(out=mv[:, 1:2], in_=mv[:, 1:2], func=AF.Sqrt,
                                 bias=eps5[:])
            nc.vector.reciprocal(mv[:, 1:2], mv[:, 1:2])
            nc.vector.tensor_scalar(x, x, mv[:, 0:1], mv[:, 1:2],
                                    op0=ALU.subtract, op1=ALU.mult)
            if g is not None:
                nc.vector.tensor_mul(x, x, g[:])
            if b is not None:
                nc.vector.tensor_add(x, x, b[:])

        _cp_flip = [0]

        def psum_copy(dst, src, rt):
            """PSUM->SBUF copy, alternating DVE/Act to balance both engines."""
            _cp_flip[0] ^= 1
            if _cp_flip[0]:
                nc.scalar.activation(out=dst, in_=src, func=AF.Copy)
            else:
                nc.vector.tensor_copy(out=dst, in_=src)

        def _idm(dt):
            return identr if dt == f32r else (identb if dt == bf16 else ident)

        def transpose_blocks(x, kn, dst, rt, dst_pre=None):
            idm = _idm(x.dtype)
            for k in range(kn):
                tp = ps_t.tile([P, P], x.dtype, tag="tp")
                nc.tensor.transpose(tp[:], x[:, k * P:(k + 1) * P], idm[:])
                d = dst[:, k, :] if dst_pre is None else dst
                if dst_pre is not None:
                    d = dst[:, dst_pre, k, :]
                psum_copy(d, tp[:], rt)

        def transpose_full(x, dst, rt):
            idm = _idm(x.dtype)
            tp = ps_t.tile([P, P], x.dtype, tag="tp")
            nc.tensor.transpose(tp[:], x[:], idm[:])
            psum_copy(dst[:], tp[:], rt)

        # ---------------- front (generator: two interleaved row-tile chains) --
        def front(rt):
            # Deferred-norm flow: each l2norm's 1/||x|| is computed as a [P,1]
            # per-row scalar IN PARALLEL with the transposes+GEMM of the raw
            # values; the scale (and bias) is applied to the GEMM result in
            # one fused scalar_tensor_tensor op (scale commutes through the
            # matmul; biases were removed from the weights).
            emb = emb_tiles[rt]
            s_emb = l2norm_scalar(emb[:], D, rt, "e")
            embT = work.tile([P, 4, P], f32r, tag=f"embT{rt}")
            transpose_blocks(emb[:], 4, embT, rt)
            yield
            ci_ps = ps_acc.tile([P, 512], f32, tag=f"acc{rt}")
            for k in range(4):
                nc.tensor.matmul(ci_ps[:, :C], lhsT=embT[:, k, :],
                                 rhs=cp_w[:, k, :], start=(k == 0), stop=(k == 3))
            # cin_v = (emb_raw @ cpW^T) * s_emb + cp_b   (C cols; rest zeroed
            # at program start)
            cin = cin_tiles[rt]
            nc.vector.scalar_tensor_tensor(
                out=cin[:, :C], in0=ci_ps[:, :C], scalar=s_emb[:],
                in1=cp_b[:], op0=ALU.mult, op1=ALU.add)
            # normalize emb in place for the fused stage (after transposes);
            # gpsimd: off the busy DVE
            nc.gpsimd.tensor_scalar_mul(emb[:], emb[:], s_emb[:])
            yield
            s_cin = l2norm_scalar(cin[:, :C], C, rt, "c")
            cinT = work.tile([P, P], f32r, tag=f"cinT{rt}")
            transpose_full(cin, cinT, rt)
            yield
            h1 = work.tile([P, H], bf16, tag=f"h1_{rt}")
            for half in range(2):
                hp = ps_acc.tile([P, 512], f32, tag=f"acc{rt}")
                nc.tensor.matmul(hp[:], lhsT=cinT[:],
                                 rhs=h1_w[:, half * 512:(half + 1) * 512],
                                 start=True, stop=True)
                sl = slice(half * 512, (half + 1) * 512)
                nc.vector.scalar_tensor_tensor(
                    out=h1[:, sl], in0=hp[:], scalar=s_cin[:],
                    in1=h1_b[:, sl], op0=ALU.mult, op1=ALU.add)
                nc.vector.tensor_scalar_max(h1[:, sl], h1[:, sl], 0.0)
                yield
            h1T = work.tile([P, 8, P], bf16, tag=f"h1T{rt}")
            transpose_blocks(h1[:], 8, h1T, rt)
            yield
            co_ps = ps_acc.tile([P, 512], f32, tag=f"acc{rt}")
            for k in range(8):
                nc.tensor.matmul(co_ps[:, :C], lhsT=h1T[:, k, :],
                                 rhs=h2_w[:, k, :], start=(k == 0), stop=(k == 7))
            cout = cout_tiles[rt]
            nc.vector.tensor_add(out=cout[:, :C], in0=co_ps[:, :C], in1=h2_b[:])
            yield
            s_cout = l2norm_scalar(cout[:, :C], C, rt, "o")
            coutT = work.tile([P, P], f32r, tag=f"coutT{rt}")
            transpose_full(cout, coutT, rt)
            yield
            cu_ps = ps_acc.tile([P, 512], f32, tag=f"acc{rt}")
            nc.tensor.matmul(cu_ps[:], lhsT=coutT[:], rhs=up_w[:],
                             start=True, stop=True)
            ht = work.tile([P, D], f32, tag=f"ht{rt}")
            nc.vector.scalar_tensor_tensor(
                out=ht[:], in0=cu_ps[:], scalar=s_cout[:],
                in1=up_b[:], op0=ALU.mult, op1=ALU.add)
            yield
            # NOTE: reference applies l2norm(core_up) then LN; LN is invariant
            # to per-row scaling, so that l2norm is dropped (h_prev == 0).
            layernorm_inplace(ht[:], D, rt, rn_g, rn_b)
            yield
            fused = work.tile([P, 4 * D], f32, tag=f"fused{rt}")
            nc.vector.tensor_copy(out=fused[:, 0:D], in_=emb[:])
            nc.vector.tensor_copy(out=fused[:, D:2 * D], in_=ht[:])
            nc.gpsimd.tensor_mul(out=fused[:, 2 * D:3 * D], in0=emb[:], in1=ht[:])
            nc.gpsimd.tensor_sub(out=fused[:, 3 * D:4 * D], in0=emb[:], in1=ht[:])
            yield
            layernorm_inplace(fused[:], 4 * D, rt)
            yield
            fusedT = work.tile([P, 16, P], bf16, tag=f"fusedT{rt}")
            transpose_blocks(fused[:], 16, fusedT, rt)
            yield
            x1a = ps_acc.tile([P, 512], f32, tag=f"acc{rt}")
            x1b = ps_acc.tile([P, 512], f32, tag=f"accb{rt}")
            for k in range(16):
                nc.tensor.matmul(x1a[:], lhsT=fusedT[:, k, :], rhs=p1_w[:, k, 0:512],
                                 start=(k == 0), stop=(k == 15))
                nc.tensor.matmul(x1b[:], lhsT=fusedT[:, k, :], rhs=p1_w[:, k, 512:H],
                                 start=(k == 0), stop=(k == 15))
            xg = work.tile([P, H], f32, tag=f"xg{rt}")
            nc.vector.tensor_add(out=xg[:, 0:512], in0=x1a[:], in1=p1_b[:, 0:512])
            nc.vector.tensor_add(out=xg[:, 512:H], in0=x1b[:], in1=p1_b[:, 512:H])
            yield
            nc.scalar.activation(out=xg[:], in_=xg[:], func=AF.Gelu)
            yield
            layernorm_inplace(xg[:], H, rt)
            yield
            xg_b = work.tile([P, H], bf16, tag=f"xgb{rt}")
            nc.vector.tensor_copy(out=xg_b[:], in_=xg[:])
            yield
            xgT = work.tile([P, 8, P], bf16, tag=f"xgT{rt}")
            transpose_blocks(xg_b[:], 8, xgT, rt)
            yield
            x2_ps = ps_acc.tile([P, 512], f32, tag=f"acc{rt}")
            for k in range(8):
                nc.tensor.matmul(x2_ps[:], lhsT=xgT[:, k, :], rhs=p2_w[:, k, :],
                                 start=(k == 0), stop=(k == 7))
            xo = work.tile([P, D], f32, tag=f"xo{rt}")
            nc.vector.tensor_add(out=xo[:], in0=x2_ps[:], in1=p2_b[:])
            nc.vector.tensor_add(out=xo[:], in0=xo[:], in1=emb[:])
            yield
            layernorm_inplace(xo[:], D, rt)
            yield
            transpose_blocks(xo[:], 4, outTex, rt, dst_pre=rt)

        gens = [front(rt) for rt in range(RT_N)]
        live = list(gens)
        while live:
            nxt = []
            for g in live:
                try:
                    next(g)
                    nxt.append(g)
                except StopIteration:
                    pass
            live = nxt

        # ---------------- exchange (pairwise AllGather of packed outT) -------
        nc.sync.dma_start(out=gin, in_=outTex[:])
        nc.gpsimd.collective_compute(
            "AllGather", mybir.AluOpType.bypass,
            replica_groups=[[0, 1], [2, 3], [4, 5], [6, 7]],
            ins=[gin], outs=[gbuf],
        )
        # pouT = both gather slots [slot(2), rt(2)] -> 4 lm row tiles in pair
        # order [lo0, lo1, hi0, hi1]; identical on both pair members.
        nc.gpsimd.dma_start(
            out=pouT[:].rearrange("p (t r) k m -> p t r k m", t=2),
            in_=gbuf.rearrange("t p (r k m) -> p t r k m", k=4, m=P))

        # ---------------- lm_head -------------------------------------------
        # Phase A: own rows (outTex) x prefix tiles of BOTH halves -> outA.
        # Phase B: all pair rows (pouT slots) x remaining own-half tiles -> outB.
        outA_r = outA_ap.rearrange("(r p) v -> p r v", p=P)
        outB_r = outB_ap.rearrange("(r p) v -> p r v", p=P)

        copy_engines = [nc.vector, nc.scalar]
        ci = 0

        def lm_tile(wt, lhsTs, ls):
            nonlocal ci
            for i, lt in enumerate(lhsTs):
                for j in range(2):
                    lp = ps_lm.tile([P, VT], f32, tag="lm")
                    for k in range(4):
                        nc.tensor.matmul(lp[:], lhsT=lt[:, k, :],
                                         rhs=wt[:, 4 * j + k, :],
                                         start=(k == 0), stop=(k == 3))
                    eng = copy_engines[ci % len(copy_engines)]
                    ci += 1
                    if eng is nc.scalar:
                        eng.activation(out=ls[:, i, j, :], in_=lp[:], func=AF.Copy)
                    else:
                        eng.tensor_copy(out=ls[:, i, j, :], in_=lp[:])

        own_lhsTs = [outTex[:, r, :, :] for r in range(RT_N)]
        all_lhsTs = [pouT[:, r, :, :] for r in range(RT_ALL)]

        # Phase A: tiles [0..pre_k-1] (own half) then [nv2c..nv2c+pre_k-1]
        # (other half's prefix); write outA[:, hx, i, :]
        for ai in range(2 * pre_k):
            hx, i = divmod(ai, pre_k)
            if ai < len(wt_pre):
                wt = wt_pre[ai]
            else:
                wt = lmw.tile([P, 8, VT], LM_DT, tag="wt")
                nc.sync.dma_start(out=wt[:],
                                  in_=a["wv"][_pre_tile_idx(ai, nv2c, pre_k)])
            ls = lms.tile([P, RT_N, 2, VT], bf16, tag="lsA")
            lm_tile(wt, own_lhsTs, ls)
            col = ai * 2 * VT
            nc.sync.dma_start(out=outA_r[:, :, col:col + 2 * VT], in_=ls[:])

        # Phase B: tiles [pre_k..nv2c-1], all four row tiles
        for vt2 in range(pre_k, nv2c):
            wt = lmw.tile([P, 8, VT], LM_DT, tag="wt")
            nc.sync.dma_start(out=wt[:], in_=a["wv"][vt2])
            ls = lms.tile([P, RT_ALL, 2, VT], bf16, tag="lsB")
            lm_tile(wt, all_lhsTs, ls)
            col = (vt2 - pre_k) * 2 * VT
            nc.sync.dma_start(out=outB_r[:, :, col:col + 2 * VT], in_=ls[:])


# ----------------------------------------------------------------------------
# Host side
# ----------------------------------------------------------------------------

_NC_CACHE = {}
LAST_RUN = None


def get_nc(nv2c=NV2C, pre_k=PRE_K):
    key = (nv2c, pre_k)
    if key not in _NC_CACHE:
        _NC_CACHE[key] = build_nc(nv2c, pre_k)
    return _NC_CACHE[key]


def prep_weights(inputs, nv2c=NV2C):
    """Host-side layout transforms. Returns (shared, wv_halves[2])."""
    ldt = np_dt(LM_DT)
    bdt = np_dt(bf16)
    f = np.float32
    emb = np.ascontiguousarray(inputs["embedding"], dtype=f)       # [V, D]
    vp = nv2c * 2 * 2 * VT

    o_g = np.asarray(inputs["out_g"], f)
    embp = np.zeros((vp, D), dtype=f)
    n = min(vp, V)
    embp[:n] = emb[:n]
    embw = embp * o_g[None, :]
    # wv[vt2, p, kk, n]: kk = sub*4 + k -> embw[vt2*1024 + sub*512 + n, k*128 + p]
    nv2t = 2 * nv2c
    wv_all = np.ascontiguousarray(
        embw.reshape(nv2t, 2, VT, 4, P).transpose(0, 4, 1, 3, 2).reshape(
            nv2t, P, 8, VT), dtype=ldt)
    pre_k = PRE_K
    wv_halves = [
        np.ascontiguousarray(np.concatenate(
            [wv_all[h * nv2c:(h + 1) * nv2c],
             wv_all[(1 - h) * nv2c:(1 - h) * nv2c + pre_k]], axis=0))
        for h in range(2)]

    def t_tiles(w_t, kn, nn, dt=f):
        return np.ascontiguousarray(
            w_t.reshape(kn, P, nn).transpose(1, 0, 2), dtype=dt)

    cp_w = t_tiles(inputs["core_proj_w"].T.astype(f), 4, C)

    h1_w = np.zeros((P, H), dtype=f)
    h1_w[:C] = inputs["h_w1"].T

    h2_w = t_tiles(inputs["h_w2"].T.astype(f), 8, C, bdt)

    up_w = np.zeros((P, D), dtype=f)
    up_w[:C] = inputs["up_w"].T

    fu_g = np.asarray(inputs["fusion_g"], f); fu_b = np.asarray(inputs["fusion_b"], f)
    p1W = inputs["p1_w"].T.astype(f)
    p1Wg = fu_g[:, None] * p1W
    p1_bf = np.asarray(inputs["p1_b"], f) + fu_b @ p1W
    p1_w = t_tiles(p1Wg, 16, H, bdt)

    pl_g = np.asarray(inputs["pln_g"], f); pl_b = np.asarray(inputs["pln_b"], f)
    p2W = inputs["p2_w"].T.astype(f)
    p2Wg = pl_g[:, None] * p2W
    p2_bf = np.asarray(inputs["p2_b"], f) + pl_b @ p2W
    p2_w = t_tiles(p2Wg, 8, D, bdt)

    shared = {
        "etab": emb,
        "cp_w": cp_w,
        "cp_b": np.asarray(inputs["core_proj_b"], dtype=f),
        "h1_w": h1_w,
        "h2_w": h2_w,
        "h2_b": np.asarray(inputs["h_b2"], dtype=f),
        "up_w": up_w,
        "h1_b": np.asarray(inputs["h_b1"], dtype=f),
        "up_b": np.asarray(inputs["up_b"], dtype=f),
        "rn_g": np.asarray(inputs["r_norm_g"], dtype=f),
        "rn_b": np.asarray(inputs["r_norm_b"], dtype=f),
        "p1_w": p1_w,
        "p1_b": p1_bf,
        "p2_w": p2_w,
        "p2_b": p2_bf,
    }
    assert not np.any(np.asarray(inputs["out_b"])), "out_b nonzero: host fallback"
    return shared, wv_halves


def run_device(inputs, nv2c=NV2C, pre_k=PRE_K, trace=False):
    global LAST_RUN
    shared, wv_halves = prep_weights(inputs, nv2c)
    nc = get_nc(nv2c, pre_k)
    tok = np.asarray(inputs["token_ids"]).astype(np.int32).reshape(
        NCORES, BB_FULL, 1)
    in_maps = [dict(shared, tok=np.ascontiguousarray(tok[c]),
                    wv=wv_halves[c % 2])
               for c in range(NCORES)]
    res = run_bass_kernel_spmd(nc, in_maps, list(range(NCORES)), trace=trace)
    LAST_RUN = res
    vc = nv2c * 2 * VT
    pk = pre_k * 2 * VT
    out = np.zeros((B, 2 * vc), dtype=np.float32)
    for c in range(NCORES):
        oA = res.results[c]["outA"].astype(np.float32)   # [256, 2*pk]
        oB = res.results[c]["outB"].astype(np.float32)   # [512, vc-pk]
        pair, h = divmod(c, 2)
        own_rows = slice(c * 256, (c + 1) * 256)
        pair_rows = slice(pair * 512, (pair + 1) * 512)
        out[own_rows, h * vc:h * vc + pk] = oA[:, :pk]
        out[own_rows, (1 - h) * vc:(1 - h) * vc + pk] = oA[:, pk:]
        out[pair_rows, h * vc + pk:(h + 1) * vc] = oB
    return out[:, :V]


def _ref_numpy(token_ids, h_prev, R_weight, embedding, core_proj_w, core_proj_b,
               h_w1, h_b1, h_w2, h_b2, up_w, up_b, r_norm_g, r_norm_b,
               fusion_g, fusion_b, p1_w, p1_b, pln_g, pln_b, p2_w, p2_b,
               out_g, out_b):
    """Exact-math fallback (only used if h_prev or out_b is nonzero)."""
    from math import erf
    f = np.float32
    ALPHA, R_DECAY, ETA_R_LOCAL, SURPRISE = 0.1, 0.999, 0.002, 1.0

    def l2n(x):
        return x / np.maximum(np.linalg.norm(x, axis=-1, keepdims=True), 1e-12)

    def ln(x, g, b):
        m = x.mean(-1, keepdims=True)
        v = x.var(-1, keepdims=True)
        return (x - m) / np.sqrt(v + 1e-5) * g + b

    emb = l2n(embedding[token_ids].astype(f))
    core_in = l2n(emb @ core_proj_w.T + core_proj_b)
    h1 = np.maximum(core_in @ h_w1.T + h_b1, 0)
    core_out = l2n(h1 @ h_w2.T + h_b2)
    core_up = l2n(core_out @ up_w.T + up_b)
    x_hat = h_prev @ R_weight
    eps = core_up - x_hat
    dR = h_prev.T @ eps / h_prev.shape[0]
    R_new = np.clip(R_DECAY * R_weight + ETA_R_LOCAL * SURPRISE * dR, -3.0, 3.0)
    temporal = h_prev @ R_new
    h_t = ln(core_up + ALPHA * temporal, r_norm_g, r_norm_b)
    fused = np.concatenate([emb, h_t, emb * h_t, emb - h_t], axis=-1)
    fused = ln(fused, fusion_g, fusion_b)
    x = fused @ p1_w.T + p1_b
    x = x * 0.5 * (1.0 + np.vectorize(erf)(x / np.sqrt(2.0)).astype(f))
    x = ln(x, pln_g, pln_b)
    x = x @ p2_w.T + p2_b
    out = ln(x + emb, out_g, out_b)
    return (out @ embedding.T).astype(f)


def kernel(**inputs):
    if np.any(np.asarray(inputs["h_prev"])) or np.any(np.asarray(inputs["out_b"])):
        return _ref_numpy(**{k: np.asarray(v) for k, v in inputs.items()})
    return run_device(inputs)


if __name__ == "__main__":
    nc = build_nc(nv2c=2, pre_k=1)
    print("built ok:", nc)


# revision 37
# speedup vs baseline: 1.0314x; 1.0314x over previous
"""Trainium2 Bass kernel for nn_DecoupledEmbeddingModel (B=2048, D=512, C=110, H=1024, V=50257).

Strategy: 8 cores, paired. Each core runs the front MLP stack for its own 256
rows, then the pair exchanges final activations (pairwise AllGather, 0.5 MB)
and each pair member computes the tied lm_head for all 512 pair rows against
HALF the vocab (25600 entries). This halves the dominant lm weight streaming
vs. pure data-parallel while keeping the PE tensor engine the bottleneck
(dense matmul stream -> full 2.4 GHz p-state).

The lm loop covers the exchange latency with a phase A that computes the
core's OWN rows for the first PRE_K vocab tiles of BOTH halves (pair symmetry
makes coverage exact with zero redundancy), then phase B computes all four
pair row tiles for the remaining own-half tiles.

The delta-rule fast-weight path (x_hat / dR / temporal) scales with h_prev,
which setup_inputs always provides as zeros; a full-math host fallback
handles the general case.
"""

import math
import sys

sys.path.insert(0, "/opt/trn_rl_repo")

import numpy as np

import concourse.bass as bass
import concourse.tile as tile
from concourse import bacc, mybir
from concourse.bass_utils import run_bass_kernel_spmd
from concourse.masks import make_identity

AF = mybir.ActivationFunctionType
ALU = mybir.AluOpType
f32 = mybir.dt.float32
f32r = mybir.dt.float32r
bf16 = mybir.dt.bfloat16
P = 128

# Model dims
V, D, C, H = 50257, 512, 110, 1024
B = 2048
NCORES = 8

# Config knobs
LM_DT = mybir.dt.bfloat16
VT = 512                      # vocab tile width
LMW_BUFS = 4
LS_BUFS = 2
PRE_K = 5                     # own-rows-only prefix (covers exchange latency)

# Vocab padded to pairs of 512 tiles, split across 2 cores
NV2_FULL = math.ceil(V / (2 * VT))    # 50 double-tiles total
NV2C = NV2_FULL // 2                  # 25 double-tiles per core
VP_FULL = NV2_FULL * 2 * VT           # 51200
VC = NV2C * 2 * VT                    # 25600 vocab per core
BB_FULL = B // NCORES                 # 256 own rows
RT_N = BB_FULL // P                   # 2 own row tiles
RT_ALL = 2 * RT_N                     # 4 row tiles in lm (own + partner)


def np_dt(dt):
    return np.dtype(mybir.dt.np(dt))


# ----------------------------------------------------------------------------
# Device program
# ----------------------------------------------------------------------------

def build_nc(nv2c=NV2C, pre_k=PRE_K):
    nc = bacc.Bacc("TRN2", target_bir_lowering=False, debug=False,
                   enable_asserts=False, num_devices=NCORES)

    a = {}
    def din(name, shape, dt=f32):
        a[name] = nc.dram_tensor(name, list(shape), dt, kind="ExternalInput").ap()

    din("tok", [BB_FULL, 1], mybir.dt.int32)
    din("etab", [V, D])
    # wv tiles: [0..nv2c-1] own vocab half, [nv2c..nv2c+pre_k-1] prefix of the
    # OTHER half (phase A covers own rows for both halves' prefixes; pair
    # symmetry makes coverage exact with no redundancy)
    din("wv", [nv2c + pre_k, P, 8, VT], LM_DT)
    din("cp_w", [P, 4, C], f32r)
    din("cp_b", [C])
    din("h1_w", [P, H], f32r)      # rows: 0..109 = h_w1.T, 110 = h_b1, rest 0
    din("h2_w", [P, 8, C], bf16)
    din("h2_b", [C])
    din("up_w", [P, D], f32r)      # rows: 0..109 = up_w.T, 110 = up_b, rest 0
    din("h1_b", [H]); din("up_b", [D])
    din("rn_g", [D]); din("rn_b", [D])
    din("p1_w", [P, 16, H], bf16)  # fusion-LN g/b folded in
    din("p1_b", [H])
    din("p2_w", [P, 8, D], bf16)   # pln-LN g/b folded in
    din("p2_b", [D])
    # exchange buffers (DRAM): own packed outT -> pair AllGather
    dscr = nc.dram_tensor("dscr", [P, 1], f32r).ap()
    gin = nc.dram_tensor("gin", [P, RT_N * 4 * P], bf16).ap()
    gbuf = nc.dram_tensor("gbuf", [2, P, RT_N * 4 * P], bf16).ap()
    # outA: own rows x prefix tiles of both halves; outB: all pair rows x rest
    outA_ap = nc.dram_tensor("outA", [RT_N * P, 2 * pre_k * 2 * VT], bf16,
                             kind="ExternalOutput").ap()
    outB_ap = nc.dram_tensor("outB", [RT_ALL * P, (nv2c - pre_k) * 2 * VT],
                             bf16, kind="ExternalOutput").ap()

    with tile.TileContext(nc) as tc:
        _program(tc, a, dscr, gin, gbuf, outA_ap, outB_ap, nv2c, pre_k)
    nc.compile()
    return nc


def _pre_tile_idx(ai, nv2c, pre_k):
    """Phase-A tile order: own-half prefix then other-half prefix."""
    hx, i = divmod(ai, pre_k)
    return i if hx == 0 else nv2c + i


def _program(tc, a, dscr, gin, gbuf, outA_ap, outB_ap, nv2c, pre_k):
    nc = tc.nc
    from contextlib import ExitStack
    ctx = ExitStack()
    with ctx:
        consts = ctx.enter_context(tc.tile_pool(name="consts", bufs=1))
        work = ctx.enter_context(tc.tile_pool(name="work", bufs=1))
        lmw = ctx.enter_context(tc.tile_pool(name="lmw", bufs=LMW_BUFS))
        lms = ctx.enter_context(tc.tile_pool(name="lms", bufs=LS_BUFS))
        ps_acc = ctx.enter_context(tc.tile_pool(name="ps_acc", bufs=1, space="PSUM"))
        ps_t = ctx.enter_context(tc.tile_pool(name="ps_t", bufs=2, space="PSUM"))
        ps_lm = ctx.enter_context(tc.tile_pool(name="ps_lm", bufs=2, space="PSUM"))

        ident = consts.tile([P, P], f32)
        make_identity(nc, ident[:])
        identr = consts.tile([P, P], f32r)
        nc.vector.tensor_copy(out=identr[:], in_=ident[:])
        identb = consts.tile([P, P], bf16)
        nc.vector.tensor_copy(out=identb[:], in_=ident[:])
        eps5 = consts.tile([P, 1], f32)
        nc.vector.memset(eps5[:], 1e-5)

        def brep(name, n):
            t = consts.tile([P, n], f32, tag=f"br_{name}")
            src = a[name]
            bsrc = bass.AP(tensor=src.tensor, offset=src.offset,
                           ap=[[0, P]] + list(src.ap))
            nc.gpsimd.dma_start(out=t[:], in_=bsrc)
            return t

        # cin/cout tiles: zero the pad columns once, off the critical path
        cin_tiles, cout_tiles = [], []
        for rt in range(RT_N):
            cin = work.tile([P, P], f32, tag=f"cin{rt}")
            nc.vector.memset(cin[:, C:], 0.0)
            cin_tiles.append(cin)
            cout = work.tile([P, P], f32, tag=f"cout{rt}")
            nc.vector.memset(cout[:, C:], 0.0)
            cout_tiles.append(cout)

        # token fetch + embedding gathers first: they gate the front and do
        # not contend with the big weight loads
        emb_tiles = []
        for rt in range(RT_N):
            tokt = work.tile([P, 1], mybir.dt.int32, tag=f"tok{rt}")
            nc.sync.dma_start(out=tokt[:], in_=a["tok"][rt * P:(rt + 1) * P, :])
            emb = work.tile([P, D], f32r, tag=f"emb{rt}")
            nc.gpsimd.indirect_dma_start(
                out=emb[:], out_offset=None,
                in_=a["etab"][:],
                in_offset=bass.IndirectOffsetOnAxis(ap=tokt[:, :1], axis=0),
            )
            emb_tiles.append(emb)
        # dummy sync-queue DMA reading the last gather: blocks the big weight
        # loads below from hogging the (exclusive) DMA engines before the
        # latency-critical gathers have landed
        nc.sync.dma_start(out=dscr, in_=emb_tiles[-1][:, 0:1])

        cp_b = brep("cp_b", C)
        h2_b = brep("h2_b", C)
        h1_b = brep("h1_b", H)
        up_b = brep("up_b", D)
        rn_g = brep("rn_g", D); rn_b = brep("rn_b", D)
        p1_b = brep("p1_b", H)
        p2_b = brep("p2_b", D)

        def wload(name, shape, dt=f32r):
            t = consts.tile(list(shape), dt, tag=f"w_{name}")
            nc.sync.dma_start(out=t[:], in_=a[name])
            return t

        cp_w = wload("cp_w", [P, 4, C])
        h1_w = wload("h1_w", [P, H])
        h2_w = wload("h2_w", [P, 8, C], bf16)
        up_w = wload("up_w", [P, D])
        p2_w = wload("p2_w", [P, 8, D], bf16)
        p1_w = wload("p1_w", [P, 16, H], bf16)
        # prefetch the first lm weight tiles while the front runs (the sync
        # queue later blocks on front-dependent DMAs)
        wt_pre = []
        for i in range(LMW_BUFS):
            wt = lmw.tile([P, 8, VT], LM_DT, tag="wt")
            nc.sync.dma_start(out=wt[:], in_=a["wv"][_pre_tile_idx(i, nv2c, pre_k)])
            wt_pre.append(wt)

        # packed own outT (exchange payload + own lm lhsT): [p, rt, k, m]
        outTex = consts.tile([P, RT_N, 4, P], LM_DT, tag="outTex", name="outTex")
        # both pair members' outT read back from the gather: [p, slot*rt, k, m]
        pouT = consts.tile([P, RT_ALL, 4, P], LM_DT, tag="pouT", name="pouT")

        # ---------------- helpers ----------------
        def l2norm_scalar(x, n, rt, tag):
            """Return [P,1] tile = 1/||x||_2 (no clamp; rows never zero)."""
            sq = work.tile([P, n], f32, tag=f"sq{n}_{rt}")
            ss = work.tile([P, 1], f32, tag=f"ss_{tag}{rt}")
            nc.scalar.activation(out=sq[:], in_=x, func=AF.Square,
                                 accum_out=ss[:])
            nc.scalar.activation(out=ss[:], in_=ss[:], func=AF.Sqrt)
            nc.vector.reciprocal(ss[:], ss[:])
            return ss

        def layernorm_inplace(x, n, rt, g=None, b=None):
            nsub = n // 512
            st = work.tile([P, nsub, 6], f32, tag=f"lnst{rt}")
            for i in range(nsub):
                nc.vector.bn_stats(out=st[:, i, :], in_=x[:, i * 512:(i + 1) * 512])
            mv = work.tile([P, 2], f32, tag=f"lnmv{rt}")
            nc.vector.bn_aggr(out=mv[:], in_=st[:])
            nc.scalar.activation(out=mv[:, 1:2], in_=mv[:, 1:2], func=AF.Sqrt,
                                 bias=eps5[:])
            nc.vector.reciprocal(mv[:, 1:2], mv[:, 1:2])
            nc.vector.tensor_scalar(x, x, mv[:, 0:1], mv[:, 1:2],
                                    op0=ALU.subtract, op1=ALU.mult)
            if g is not None:
                nc.vector.tensor_mul(x, x, g[:])
            if b is not None:
                nc.vector.tensor_add(x, x, b[:])

        _cp_flip = [0]

        def psum_copy(dst, src, rt):
            """PSUM->SBUF copy, alternating DVE/Act to balance both engines."""
            _cp_flip[0] ^= 1
            if _cp_flip[0]:
                nc.scalar.activation(out=dst, in_=src, func=AF.Copy)
            else:
                nc.vector.tensor_copy(out=dst, in_=src)

        def _idm(dt):
            return identr if dt == f32r else (identb if dt == bf16 else ident)

        def transpose_blocks(x, kn, dst, rt, dst_pre=None):
            idm = _idm(x.dtype)
            for k in range(kn):
                tp = ps_t.tile([P, P], x.dtype, tag="tp")
                nc.tensor.transpose(tp[:], x[:, k * P:(k + 1) * P], idm[:])
                d = dst[:, k, :] if dst_pre is None else dst
                if dst_pre is not None:
                    d = dst[:, dst_pre, k, :]
                psum_copy(d, tp[:], rt)

        def transpose_full(x, dst, rt):
            idm = _idm(x.dtype)
            tp = ps_t.tile([P, P], x.dtype, tag="tp")
            nc.tensor.transpose(tp[:], x[:], idm[:])
            psum_copy(dst[:], tp[:], rt)

        # ---------------- front (generator: two interleaved row-tile chains) --
        def front(rt):
            # Deferred-norm flow: each l2norm's 1/||x|| is computed as a [P,1]
            # per-row scalar IN PARALLEL with the transposes+GEMM of the raw
            # values; the scale (and bias) is applied to the GEMM result in
            # one fused scalar_tensor_tensor op (scale commutes through the
            # matmul; biases were removed from the weights).
            emb = emb_tiles[rt]
            s_emb = l2norm_scalar(emb[:], D, rt, "e")
            embT = work.tile([P, 4, P], f32r, tag=f"embT{rt}")
            transpose_blocks(emb[:], 4, embT, rt)
            yield
            ci_ps = ps_acc.tile([P, 512], f32, tag=f"acc{rt}")
            for k in range(4):
                nc.tensor.matmul(ci_ps[:, :C], lhsT=embT[:, k, :],
                                 rhs=cp_w[:, k, :], start=(k == 0), stop=(k == 3))
            # cin_v = (emb_raw @ cpW^T) * s_emb + cp_b   (C cols; rest zeroed
            # at program start)
            cin = cin_tiles[rt]
            nc.vector.scalar_tensor_tensor(
                out=cin[:, :C], in0=ci_ps[:, :C], scalar=s_emb[:],
                in1=cp_b[:], op0=ALU.mult, op1=ALU.add)
            # normalize emb in place for the fused stage (after transposes);
            # gpsimd: off the busy DVE
            nc.gpsimd.tensor_scalar_mul(emb[:], emb[:], s_emb[:])
            yield
            s_cin = l2norm_scalar(cin[:, :C], C, rt, "c")
            cinT = work.tile([P, P], f32r, tag=f"cinT{rt}")
            transpose_full(cin, cinT, rt)
            yield
            h1 = work.tile([P, H], bf16, tag=f"h1_{rt}")
            for half in range(2):
                hp = ps_acc.tile([P, 512], f32, tag=f"acc{rt}")
                nc.tensor.matmul(hp[:], lhsT=cinT[:],
                                 rhs=h1_w[:, half * 512:(half + 1) * 512],
                                 start=True, stop=True)
                sl = slice(half * 512, (half + 1) * 512)
                nc.vector.scalar_tensor_tensor(
                    out=h1[:, sl], in0=hp[:], scalar=s_cin[:],
                    in1=h1_b[:, sl], op0=ALU.mult, op1=ALU.add)
                nc.vector.tensor_scalar_max(h1[:, sl], h1[:, sl], 0.0)
                yield
            h1T = work.tile([P, 8, P], bf16, tag=f"h1T{rt}")
            transpose_blocks(h1[:], 8, h1T, rt)
            yield
            co_ps = ps_acc.tile([P, 512], f32, tag=f"acc{rt}")
            for k in range(8):
                nc.tensor.matmul(co_ps[:, :C], lhsT=h1T[:, k, :],
                                 rhs=h2_w[:, k, :], start=(k == 0), stop=(k == 7))
            cout = cout_tiles[rt]
            nc.vector.tensor_add(out=cout[:, :C], in0=co_ps[:, :C], in1=h2_b[:])
            yield
            s_cout = l2norm_scalar(cout[:, :C], C, rt, "o")
            coutT = work.tile([P, P], f32r, tag=f"coutT{rt}")
            transpose_full(cout, coutT, rt)
            yield
            cu_ps = ps_acc.tile([P, 512], f32, tag=f"acc{rt}")
            nc.tensor.matmul(cu_ps[:], lhsT=coutT[:], rhs=up_w[:],
                             start=True, stop=True)
            ht = work.tile([P, D], f32, tag=f"ht{rt}")
            nc.vector.scalar_tensor_tensor(
                out=ht[:], in0=cu_ps[:], scalar=s_cout[:],
                in1=up_b[:], op0=ALU.mult, op1=ALU.add)
            yield
            # NOTE: reference applies l2norm(core_up) then LN; LN is invariant
            # to per-row scaling, so that l2norm is dropped (h_prev == 0).
            layernorm_inplace(ht[:], D, rt, rn_g, rn_b)
            yield
            fused = work.tile([P, 4 * D], f32, tag=f"fused{rt}")
            nc.vector.tensor_copy(out=fused[:, 0:D], in_=emb[:])
            nc.vector.tensor_copy(out=fused[:, D:2 * D], in_=ht[:])
            nc.gpsimd.tensor_mul(out=fused[:, 2 * D:3 * D], in0=emb[:], in1=ht[:])
            nc.gpsimd.tensor_sub(out=fused[:, 3 * D:4 * D], in0=emb[:], in1=ht[:])
            yield
            layernorm_inplace(fused[:], 4 * D, rt)
            yield
            fusedT = work.tile([P, 16, P], bf16, tag=f"fusedT{rt}")
            transpose_blocks(fused[:], 16, fusedT, rt)
            yield
            x1a = ps_acc.tile([P, 512], f32, tag=f"acc{rt}")
            x1b = ps_acc.tile([P, 512], f32, tag=f"accb{rt}")
            for k in range(16):
                nc.tensor.matmul(x1a[:], lhsT=fusedT[:, k, :], rhs=p1_w[:, k, 0:512],
                                 start=(k == 0), stop=(k == 15))
                nc.tensor.matmul(x1b[:], lhsT=fusedT[:, k, :], rhs=p1_w[:, k, 512:H],
                                 start=(k == 0), stop=(k == 15))
            xg = work.tile([P, H], f32, tag=f"xg{rt}")
            nc.vector.tensor_add(out=xg[:, 0:512], in0=x1a[:], in1=p1_b[:, 0:512])
            nc.vector.tensor_add(out=xg[:, 512:H], in0=x1b[:], in1=p1_b[:, 512:H])
            yield
            nc.scalar.activation(out=xg[:], in_=xg[:], func=AF.Gelu)
            yield
            layernorm_inplace(xg[:], H, rt)
            yield
            xg_b = work.tile([P, H], bf16, tag=f"xgb{rt}")
            nc.vector.tensor_copy(out=xg_b[:], in_=xg[:])
            yield
            xgT = work.tile([P, 8, P], bf16, tag=f"xgT{rt}")
            transpose_blocks(xg_b[:], 8, xgT, rt)
            yield
            x2_ps = ps_acc.tile([P, 512], f32, tag=f"acc{rt}")
            for k in range(8):
                nc.tensor.matmul(x2_ps[:], lhsT=xgT[:, k, :], rhs=p2_w[:, k, :],
                                 start=(k == 0), stop=(k == 7))
            xo = work.tile([P, D], f32, tag=f"xo{rt}")
            nc.vector.tensor_add(out=xo[:], in0=x2_ps[:], in1=p2_b[:])
            nc.vector.tensor_add(out=xo[:], in0=xo[:], in1=emb[:])
            yield
            layernorm_inplace(xo[:], D, rt)
            yield
            transpose_blocks(xo[:], 4, outTex, rt, dst_pre=rt)

        gens = [front(rt) for rt in range(RT_N)]
        live = list(gens)
        while live:
            nxt = []
            for g in live:
                try:
                    next(g)
                    nxt.append(g)
                except StopIteration:
                    pass
            live = nxt

        # ---------------- exchange (pairwise AllGather of packed outT) -------
        nc.sync.dma_start(out=gin, in_=outTex[:])
        nc.gpsimd.collective_compute(
            "AllGather", mybir.AluOpType.bypass,
            replica_groups=[[0, 1], [2, 3], [4, 5], [6, 7]],
            ins=[gin], outs=[gbuf],
        )
        # pouT = both gather slots [slot(2), rt(2)] -> 4 lm row tiles in pair
        # order [lo0, lo1, hi0, hi1]; identical on both pair members.
        nc.gpsimd.dma_start(
            out=pouT[:].rearrange("p (t r) k m -> p t r k m", t=2),
            in_=gbuf.rearrange("t p (r k m) -> p t r k m", k=4, m=P))

        # ---------------- lm_head -------------------------------------------
        # Phase A: own rows (outTex) x prefix tiles of BOTH halves -> outA.
        # Phase B: all pair rows (pouT slots) x remaining own-half tiles -> outB.
        outA_r = outA_ap.rearrange("(r p) v -> p r v", p=P)
        outB_r = outB_ap.rearrange("(r p) v -> p r v", p=P)

        copy_engines = [nc.vector, nc.scalar]
        ci = 0

        def lm_tile(wt, lhsTs, ls):
            nonlocal ci
            for i, lt in enumerate(lhsTs):
                for j in range(2):
                    lp = ps_lm.tile([P, VT], f32, tag="lm")
                    for k in range(4):
                        nc.tensor.matmul(lp[:], lhsT=lt[:, k, :],
                                         rhs=wt[:, 4 * j + k, :],
                                         start=(k == 0), stop=(k == 3))
                    eng = copy_engines[ci % len(copy_engines)]
                    ci += 1
                    if eng is nc.scalar:
                        eng.activation(out=ls[:, i, j, :], in_=lp[:], func=AF.Copy)
                    else:
                        eng.tensor_copy(out=ls[:, i, j, :], in_=lp[:])

        own_lhsTs = [outTex[:, r, :, :] for r in range(RT_N)]
        all_lhsTs = [pouT[:, r, :, :] for r in range(RT_ALL)]

        # Phase A: tiles [0..pre_k-1] (own half) then [nv2c..nv2c+pre_k-1]
        # (other half's prefix); write outA[:, hx, i, :]
        for ai in range(2 * pre_k):
            hx, i = divmod(ai, pre_k)
            if ai < len(wt_pre):
                wt = wt_pre[ai]
            else:
                wt = lmw.tile([P, 8, VT], LM_DT, tag="wt")
                nc.sync.dma_start(out=wt[:],
                                  in_=a["wv"][_pre_tile_idx(ai, nv2c, pre_k)])
            ls = lms.tile([P, RT_N, 2, VT], bf16, tag="lsA")
            lm_tile(wt, own_lhsTs, ls)
            col = ai * 2 * VT
            nc.sync.dma_start(out=outA_r[:, :, col:col + 2 * VT], in_=ls[:])

        # Phase B: tiles [pre_k..nv2c-1], all four row tiles
        for vt2 in range(pre_k, nv2c):
            wt = lmw.tile([P, 8, VT], LM_DT, tag="wt")
            nc.sync.dma_start(out=wt[:], in_=a["wv"][vt2])
            ls = lms.tile([P, RT_ALL, 2, VT], bf16, tag="lsB")
            lm_tile(wt, all_lhsTs, ls)
            col = (vt2 - pre_k) * 2 * VT
            nc.sync.dma_start(out=outB_r[:, :, col:col + 2 * VT], in_=ls[:])


# ----------------------------------------------------------------------------
# Host side
# ----------------------------------------------------------------------------

_NC_CACHE = {}
LAST_RUN = None


def get_nc(nv2c=NV2C, pre_k=PRE_K):
    key = (nv2c, pre_k)
    if key not in _NC_CACHE:
        _NC_CACHE[key] = build_nc(nv2c, pre_k)
    return _NC_CACHE[key]


def prep_weights(inputs, nv2c=NV2C):
    """Host-side layout transforms. Returns (shared, wv_halves[2])."""
    ldt = np_dt(LM_DT)
    bdt = np_dt(bf16)
    f = np.float32
    emb = np.ascontiguousarray(inputs["embedding"], dtype=f)       # [V, D]
    vp = nv2c * 2 * 2 * VT

    o_g = np.asarray(inputs["out_g"], f)
    embp = np.zeros((vp, D), dtype=f)
    n = min(vp, V)
    embp[:n] = emb[:n]
    embw = embp * o_g[None, :]
    # wv[vt2, p, kk, n]: kk = sub*4 + k -> embw[vt2*1024 + sub*512 + n, k*128 + p]
    nv2t = 2 * nv2c
    wv_all = np.ascontiguousarray(
        embw.reshape(nv2t, 2, VT, 4, P).transpose(0, 4, 1, 3, 2).reshape(
            nv2t, P, 8, VT), dtype=ldt)
    pre_k = PRE_K
    wv_halves = [
        np.ascontiguousarray(np.concatenate(
            [wv_all[h * nv2c:(h + 1) * nv2c],
             wv_all[(1 - h) * nv2c:(1 - h) * nv2c + pre_k]], axis=0))
        for h in range(2)]

    def t_tiles(w_t, kn, nn, dt=f):
        return np.ascontiguousarray(
            w_t.reshape(kn, P, nn).transpose(1, 0, 2), dtype=dt)

    cp_w = t_tiles(inputs["core_proj_w"].T.astype(f), 4, C)

    h1_w = np.zeros((P, H), dtype=f)
    h1_w[:C] = inputs["h_w1"].T

    h2_w = t_tiles(inputs["h_w2"].T.astype(f), 8, C, bdt)

    up_w = np.zeros((P, D), dtype=f)
    up_w[:C] = inputs["up_w"].T

    fu_g = np.asarray(inputs["fusion_g"], f); fu_b = np.asarray(inputs["fusion_b"], f)
    p1W = inputs["p1_w"].T.astype(f)
    p1Wg = fu_g[:, None] * p1W
    p1_bf = np.asarray(inputs["p1_b"], f) + fu_b @ p1W
    p1_w = t_tiles(p1Wg, 16, H, bdt)

    pl_g = np.asarray(inputs["pln_g"], f); pl_b = np.asarray(inputs["pln_b"], f)
    p2W = inputs["p2_w"].T.astype(f)
    p2Wg = pl_g[:, None] * p2W
    p2_bf = np.asarray(inputs["p2_b"], f) + pl_b @ p2W
    p2_w = t_tiles(p2Wg, 8, D, bdt)

    shared = {
        "etab": emb,
        "cp_w": cp_w,
        "cp_b": np.asarray(inputs["core_proj_b"], dtype=f),
        "h1_w": h1_w,
        "h2_w": h2_w,
        "h2_b": np.asarray(inputs["h_b2"], dtype=f),
        "up_w": up_w,
        "h1_b": np.asarray(inputs["h_b1"], dtype=f),
        "up_b": np.asarray(inputs["up_b"], dtype=f),
        "rn_g": np.asarray(inputs["r_norm_g"], dtype=f),
        "rn_b": np.asarray(inputs["r_norm_b"], dtype=f),
        "p1_w": p1_w,
        "p1_b": p1_bf,
        "p2_w": p2_w,
        "p2_b": p2_bf,
    }
    assert not np.any(np.asarray(inputs["out_b"])), "out_b nonzero: host fallback"
    return shared, wv_halves


def run_device(inputs, nv2c=NV2C, pre_k=PRE_K, trace=False):
    global LAST_RUN
    shared, wv_halves = prep_weights(inputs, nv2c)
    nc = get_nc(nv2c, pre_k)
    tok = np.asarray(inputs["token_ids"]).astype(np.int32).reshape(
        NCORES, BB_FULL, 1)
    in_maps = [dict(shared, tok=np.ascontiguousarray(tok[c]),
                    wv=wv_halves[c % 2])
               for c in range(NCORES)]
    res = run_bass_kernel_spmd(nc, in_maps, list(range(NCORES)), trace=trace)
    LAST_RUN = res
    vc = nv2c * 2 * VT
    pk = pre_k * 2 * VT
    out = np.zeros((B, 2 * vc), dtype=np.float32)
    for c in range(NCORES):
        oA = res.results[c]["outA"].astype(np.float32)   # [256, 2*pk]
        oB = res.results[c]["outB"].astype(np.float32)   # [512, vc-pk]
        pair, h = divmod(c, 2)
        own_rows = slice(c * 256, (c + 1) * 256)
        pair_rows = slice(pair * 512, (pair + 1) * 512)
        out[own_rows, h * vc:h * vc + pk] = oA[:, :pk]
        out[own_rows, (1 - h) * vc:(1 - h) * vc + pk] = oA[:, pk:]
        out[pair_rows, h * vc + pk:(h + 1) * vc] = oB
    return out[:, :V]


def _ref_numpy(token_ids, h_prev, R_weight, embedding, core_proj_w, core_proj_b,
               h_w1, h_b1, h_w2, h_b2, up_w, up_b, r_norm_g, r_norm_b,
               fusion_g, fusion_b, p1_w, p1_b, pln_g, pln_b, p2_w, p2_b,
               out_g, out_b):
    """Exact-math fallback (only used if h_prev or out_b is nonzero)."""
    from math import erf
    f = np.float32
    ALPHA, R_DECAY, ETA_R_LOCAL, SURPRISE = 0.1, 0.999, 0.002, 1.0

    def l2n(x):
        return x / np.maximum(np.linalg.norm(x, axis=-1, keepdims=True), 1e-12)

    def ln(x, g, b):
        m = x.mean(-1, keepdims=True)
        v = x.var(-1, keepdims=True)
        return (x - m) / np.sqrt(v + 1e-5) * g + b

    emb = l2n(embedding[token_ids].astype(f))
    core_in = l2n(emb @ core_proj_w.T + core_proj_b)
    h1 = np.maximum(core_in @ h_w1.T + h_b1, 0)
    core_out = l2n(h1 @ h_w2.T + h_b2)
    core_up = l2n(core_out @ up_w.T + up_b)
    x_hat = h_prev @ R_weight
    eps = core_up - x_hat
    dR = h_prev.T @ eps / h_prev.shape[0]
    R_new = np.clip(R_DECAY * R_weight + ETA_R_LOCAL * SURPRISE * dR, -3.0, 3.0)
    temporal = h_prev @ R_new
    h_t = ln(core_up + ALPHA * temporal, r_norm_g, r_norm_b)
    fused = np.concatenate([emb, h_t, emb * h_t, emb - h_t], axis=-1)
    fused = ln(fused, fusion_g, fusion_b)
    x = fused @ p1_w.T + p1_b
    x = x * 0.5 * (1.0 + np.vectorize(erf)(x / np.sqrt(2.0)).astype(f))
    x = ln(x, pln_g, pln_b)
    x = x @ p2_w.T + p2_b
    out = ln(x + emb, out_g, out_b)
    return (out @ embedding.T).astype(f)


def kernel(**inputs):
    if np.any(np.asarray(inputs["h_prev"])) or np.any(np.asarray(inputs["out_b"])):
        return _ref_numpy(**{k: np.asarray(v) for k, v in inputs.items()})
    return run_device(inputs)


if __name__ == "__main__":
    nc = build_nc(nv2c=2, pre_k=1)
    print("built ok:", nc)
